# revision 23
# baseline (speedup 1.0000x reference)
"""Sharded attention kernel for Trainium2 (8 NeuronCores).

Computes softmax(q @ k^T / sqrt(d) + mask) @ v for q, k, v: [8192, 128] f32,
mask: [8192, 8192] f32.

Sharding: q rows and mask rows split 8 ways (1024 rows per core); k and v are
replicated. Each core computes its row-block of the output independently; the
host concatenates the 8 row-blocks.

Per-core pipeline (scores kept in natural [n, m] layout so the mask streams
from HBM with fully contiguous DMA):
  setup: PE-transpose q, k (rounded to fp32r) into Q^T [d, n], K^T [d, m];
         build fp16 V_aug = [V | ones] laid out [128 m_loc, 64 chunk, 129].
  mm1   (PE, fp32r): S_chunk [128n, 512m] = Q^T_tile.T @ K^T_chunk  -> PSUM
  stt   (DVE):       Sm = S*scale + mask_chunk -> fp16 SBUF
  trans (PE, fp16):  4x 128x128 block transposes of Sm -> PSUM (S^T blocks)
  exp   (ACT):       P^T = exp(S^T blocks), PSUM -> SBUF fp16 (FD=2048 groups)
  mm2   (PE, fp16):  ps_o [128n, 129] += P^T_block.T @ V_aug_block
                     (ones column makes ps_o[:, 128] the softmax denominator)
  norm  (DVE):       out_tile = ps_o[:, :128] * (1 / ps_o[:, 128])

The one-time K^T production (DMA + round + PE transpose + copy) is interleaved
into the first q-tile's groups, one group ahead of use, so it overlaps the
main pipeline instead of forming a serial prologue.

Max-subtraction is skipped: scores are q.k/sqrt(128) of randn data, O(1) in
magnitude, so exp is safe in f32 and softmax is shift-invariant regardless.
The mask is streamed as bf16 (host-cast): halves the dominant HBM stream; a
bf16-rounded additive mask shifts scores by <0.4% of the mask value and is
exact for an all-zeros mask.
"""

import numpy as np

import concourse.bacc as bacc
import concourse.mybir as mybir
import concourse.tile as tile
from concourse.bass import ds, ts
from concourse.bass_utils import run_bass_kernel_spmd
from concourse.masks import make_identity

N = 8192
M = 8192
D = 128
P = 128
NCORES = 8
N_SH = N // NCORES  # q rows per core (1024)
NT = N_SH // P  # q-tiles per core (8)
MC = 512  # m-chunk width (mm1 free dim)
N_MC = M // MC  # 16
TGROUP = 2  # m-chunks per exp group
GW = MC * TGROUP  # 2048 = exp group width
N_G = M // GW  # 4 groups per q-tile
N_CH = M // P  # 64 key blocks of 128
SCALE = 1.0 / float(np.sqrt(D))

F32 = mybir.dt.float32
F32R = mybir.dt.float32r
F16 = mybir.dt.float16
BF16 = mybir.dt.bfloat16
MULT = mybir.AluOpType.mult
ADD = mybir.AluOpType.add


def build_nc():
    nc = bacc.Bacc(None, target_bir_lowering=False)
    q = nc.dram_tensor("q", [N_SH, D], F32, kind="ExternalInput")
    k = nc.dram_tensor("k", [M, D], F32, kind="ExternalInput")
    v = nc.dram_tensor("v", [M, D], F32, kind="ExternalInput")
    mask = nc.dram_tensor("mask", [N_SH, M], BF16, kind="ExternalInput")
    out = nc.dram_tensor("out", [N_SH, D], F32, kind="ExternalOutput")

    with tile.TileContext(nc) as tc:
        with (
            tc.tile_pool(name="const", bufs=1) as const_pool,
            tc.tile_pool(name="big", bufs=1) as big_pool,
            tc.tile_pool(name="stage", bufs=4) as stage_pool,
            tc.tile_pool(name="maskp", bufs=4) as mask_pool,
            tc.tile_pool(name="smp", bufs=3) as sm_pool,
            tc.tile_pool(name="ptp", bufs=2) as pt_pool,
            tc.tile_pool(name="op", bufs=2) as o_pool,
            tc.tile_pool(name="ps_s", bufs=2, space="PSUM") as ps_s_pool,
            tc.tile_pool(name="ps_t", bufs=3, space="PSUM") as ps_t_pool,
            tc.tile_pool(name="ps_o", bufs=1, space="PSUM") as ps_o_pool,
        ):
            ident_f32 = const_pool.tile([P, P], F32)
            make_identity(nc, ident_f32)
            ident_f32r = const_pool.tile([P, P], F32R)
            nc.vector.tensor_copy(ident_f32r[:], ident_f32[:])
            ident_bf = const_pool.tile([P, P], F16)
            make_identity(nc, ident_bf)

            # Per-chunk K^T / per-tile Q^T tiles: fine-grained so consumers
            # only wait on the slices they read.
            kt_t = [
                big_pool.tile([P, MC], F32R, name=f"kt{i}") for i in range(N_MC)
            ]
            qt_t = [big_pool.tile([P, P], F32R, name=f"qt{i}") for i in range(NT)]
            vaug = big_pool.tile([P, N_CH, D + 1], F16)  # [m_loc, chunk, d|1]

            # -- setup: transpose-load q via PE --
            q_nat = big_pool.tile([P, NT, P], F32)
            nc.sync.dma_start(q_nat[:], q[:].rearrange("(c p) d -> p c d", p=P))
            q_r = big_pool.tile([P, NT, P], F32R)
            nc.vector.tensor_copy(q_r[:], q_nat[:])
            for i in range(NT):
                ps = ps_s_pool.tile([P, P], F32R, tag="ps_s")
                nc.tensor.transpose(ps[:], q_r[:, i, :], ident_f32r[:])
                nc.scalar.copy(qt_t[i][:], ps[:])

            v_f32 = big_pool.tile([P, N_CH, D], F32)

            def load_kt_chunk(mc):
                k_nat = stage_pool.tile([P, 4, P], F32, tag="nat")
                nc.sync.dma_start(
                    k_nat[:],
                    k[ds(mc * MC, MC), :].rearrange("(c p) d -> p c d", p=P),
                )
                k_r = stage_pool.tile([P, 4, P], F32R, tag="natr")
                nc.vector.tensor_copy(k_r[:], k_nat[:])
                for b in range(4):
                    ps = ps_s_pool.tile([P, P], F32R, tag="ps_s")
                    nc.tensor.transpose(ps[:], k_r[:, b, :], ident_f32r[:])
                    nc.scalar.copy(kt_t[mc][:, ts(b, P)], ps[:])

            for mc0 in range(TGROUP):
                load_kt_chunk(mc0)

            # -- main loop --
            for nt in range(NT):
                ps_o = ps_o_pool.tile([P, D + 1], F32, tag="ps_o")
                for g in range(N_G):
                    if nt == 0:
                        # stage this group's V chunks: [V | ones] fp16
                        cs = slice(g * (GW // P), (g + 1) * (GW // P))
                        nc.sync.dma_start(
                            v_f32[:, cs, :],
                            v[ds(g * GW, GW), :].rearrange(
                                "(c p) d -> p c d", p=P
                            ),
                        )
                        nc.vector.tensor_copy(vaug[:, cs, 0:D], v_f32[:, cs, :])
                        nc.vector.memset(vaug[:, cs, D : D + 1], 1.0)
                    ps_t = ps_t_pool.tile([P, GW], F16)
                    ps_s = ps_s_pool.tile([P, GW], F32, tag="ps_s")
                    for j in range(TGROUP):
                        mc = g * TGROUP + j
                        nc.tensor.matmul(
                            ps_s[:, ds(j * MC, MC)],
                            qt_t[nt][:],
                            kt_t[mc][:],
                            start=True,
                            stop=True,
                        )
                    if g % 2 == 0:
                        m_tg = mask_pool.tile([P, 2 * GW], BF16, tag="m_tg")
                        nc.sync.dma_start(
                            m_tg[:], mask[ts(nt, P), ts(g // 2, 2 * GW)]
                        )
                    m_t = m_tg[:, ds((g % 2) * GW, GW)]
                    sm = sm_pool.tile([P, GW], F16)
                    nc.vector.scalar_tensor_tensor(
                        sm[:], ps_s[:], SCALE, m_t, op0=MULT, op1=ADD
                    )
                    for b in range(GW // P):
                        nc.tensor.transpose(
                            ps_t[:, ds(b * P, P)],
                            sm[:, ts(b, P)],
                            ident_bf[:],
                        )
                    p_t = pt_pool.tile([P, GW], F16)
                    nc.scalar.activation(
                        p_t[:], ps_t[:], mybir.ActivationFunctionType.Exp
                    )
                    for bb in range(GW // P):
                        cglob = g * (GW // P) + bb
                        nc.tensor.matmul(
                            ps_o[:],
                            p_t[:, ts(bb, P)],
                            vaug[:, cglob, :],
                            start=(cglob == 0),
                            stop=(cglob == N_CH - 1),
                        )
                    if nt == 0 and g + 1 < N_G:
                        # prefetch next group's K^T chunks; emitted after this
                        # group's compute so exp/stt never queue behind them
                        for j2 in range(TGROUP):
                            load_kt_chunk((g + 1) * TGROUP + j2)
                l_r = o_pool.tile([P, 1], F32, tag="lr")
                nc.vector.reciprocal(l_r[:], ps_o[:, D : D + 1])
                o_sb = o_pool.tile([P, D], F32, tag="osb")
                nc.vector.tensor_scalar(
                    o_sb[:], ps_o[:, 0:D], l_r[:], None, op0=MULT
                )
                nc.sync.dma_start(out[ts(nt, P), :], o_sb[:])

    nc.compile()
    return nc


_CACHE = {}


def _get_nc():
    if "nc" not in _CACHE:
        _CACHE["nc"] = build_nc()
    return _CACHE["nc"]


def _make_in_maps(q, k, v, mask):
    import ml_dtypes

    q = np.ascontiguousarray(np.asarray(q), dtype=np.float32)
    k = np.ascontiguousarray(np.asarray(k), dtype=np.float32)
    v = np.ascontiguousarray(np.asarray(v), dtype=np.float32)
    mask = np.asarray(mask)
    if mask.dtype != ml_dtypes.bfloat16:
        mask = mask.astype(ml_dtypes.bfloat16)
    in_maps = []
    for c in range(NCORES):
        sl = slice(c * N_SH, (c + 1) * N_SH)
        in_maps.append(
            {
                "q": q[sl],
                "k": k,
                "v": v,
                "mask": np.ascontiguousarray(mask[sl]),
            }
        )
    return in_maps


def _run(q, k, v, mask, **spmd_kwargs):
    nc = _get_nc()
    res = run_bass_kernel_spmd(
        nc, _make_in_maps(q, k, v, mask), core_ids=list(range(NCORES)), **spmd_kwargs
    )
    full = np.concatenate(
        [res.results[c]["out"] for c in range(NCORES)], axis=0
    ).astype(np.float32)
    return full, res


def kernel(q, k, v, mask):
    full, _ = _run(q, k, v, mask)
    return full


# revision 24
# speedup vs baseline: 1.0623x; 1.0623x over previous
"""Sharded attention kernel for Trainium2 (8 NeuronCores).

Computes softmax(q @ k^T / sqrt(d) + mask) @ v for q, k, v: [8192, 128] f32,
mask: [8192, 8192] f32.

Sharding: q rows and mask rows split 8 ways (1024 rows per core); k and v are
replicated. Each core computes its row-block of the output independently; the
host concatenates the 8 row-blocks.

Per-core pipeline (scores kept in natural [n, m] layout so the mask streams
from HBM with fully contiguous DMA):
  setup: PE-transpose q, k (rounded to fp32r) into Q^T [d, n], K^T [d, m];
         build fp16 V_aug = [V | ones] laid out [128 m_loc, 64 chunk, 129].
  mm1   (PE, fp32r): S_chunk [128n, 512m] = Q^T_tile.T @ K^T_chunk  -> PSUM
  stt   (DVE):       Sm = S*scale + mask_chunk -> fp16 SBUF
  trans (PE, fp16):  4x 128x128 block transposes of Sm -> PSUM (S^T blocks)
  exp   (ACT):       P^T = exp(S^T blocks), PSUM -> SBUF fp16 (FD=2048 groups)
  mm2   (PE, fp16):  ps_o [128n, 129] += P^T_block.T @ V_aug_block
                     (ones column makes ps_o[:, 128] the softmax denominator)
  norm  (DVE):       out_tile = ps_o[:, :128] * (1 / ps_o[:, 128])

The one-time K^T production (DMA + round + PE transpose + copy) is interleaved
into the first q-tile's groups, one group ahead of use, so it overlaps the
main pipeline instead of forming a serial prologue.

Max-subtraction is skipped: scores are q.k/sqrt(128) of randn data, O(1) in
magnitude, so exp is safe in f32 and softmax is shift-invariant regardless.
The mask is streamed as bf16 (host-cast): halves the dominant HBM stream; a
bf16-rounded additive mask shifts scores by <0.4% of the mask value and is
exact for an all-zeros mask.
"""

import numpy as np

import concourse.bacc as bacc
import concourse.mybir as mybir
import concourse.tile as tile
from concourse.bass import ds, ts
from concourse.bass_utils import run_bass_kernel_spmd
from concourse.masks import make_identity

N = 8192
M = 8192
D = 128
P = 128
NCORES = 8
N_SH = N // NCORES  # q rows per core (1024)
NT = N_SH // P  # q-tiles per core (8)
MC = 512  # m-chunk width (mm1 free dim)
N_MC = M // MC  # 16
TGROUP = 4  # m-chunks per exp group
GW = MC * TGROUP  # 2048 = exp group width
N_G = M // GW  # 4 groups per q-tile
N_CH = M // P  # 64 key blocks of 128
SCALE = 1.0 / float(np.sqrt(D))

F32 = mybir.dt.float32
F32R = mybir.dt.float32r
F16 = mybir.dt.float16
BF16 = mybir.dt.bfloat16
MULT = mybir.AluOpType.mult
ADD = mybir.AluOpType.add


def build_nc():
    nc = bacc.Bacc(None, target_bir_lowering=False)
    q = nc.dram_tensor("q", [N_SH, D], F32, kind="ExternalInput")
    k = nc.dram_tensor("k", [M, D], F32, kind="ExternalInput")
    v = nc.dram_tensor("v", [M, D], F32, kind="ExternalInput")
    mask = nc.dram_tensor("mask", [N_SH, M], BF16, kind="ExternalInput")
    out = nc.dram_tensor("out", [N_SH, D], F32, kind="ExternalOutput")

    with tile.TileContext(nc) as tc:
        with (
            tc.tile_pool(name="const", bufs=1) as const_pool,
            tc.tile_pool(name="big", bufs=1) as big_pool,
            tc.tile_pool(name="stage", bufs=4) as stage_pool,
            tc.tile_pool(name="maskp", bufs=4) as mask_pool,
            tc.tile_pool(name="smp", bufs=3) as sm_pool,
            tc.tile_pool(name="ptp", bufs=2) as pt_pool,
            tc.tile_pool(name="op", bufs=2) as o_pool,
            tc.tile_pool(name="ps_s", bufs=3, space="PSUM") as ps_s_pool,
            tc.tile_pool(name="ps_t", bufs=2, space="PSUM") as ps_t_pool,
            tc.tile_pool(name="ps_o", bufs=1, space="PSUM") as ps_o_pool,
        ):
            ident_bf = const_pool.tile([P, P], F16)
            make_identity(nc, ident_bf)

            # Per-chunk K^T / per-tile Q^T tiles: fine-grained so consumers
            # only wait on the slices they read.
            kt_t = [
                big_pool.tile([P, MC], F16, name=f"kt{i}") for i in range(N_MC)
            ]
            qt_t = [big_pool.tile([P, P], F16, name=f"qt{i}") for i in range(NT)]
            vaug = big_pool.tile([P, N_CH, D + 1], F16)  # [m_loc, chunk, d|1]

            # -- setup: transpose-load q via PE --
            q_nat = big_pool.tile([P, NT, P], F32)
            nc.sync.dma_start(q_nat[:], q[:].rearrange("(c p) d -> p c d", p=P))
            q_r = big_pool.tile([P, NT, P], F16)
            nc.vector.tensor_copy(q_r[:], q_nat[:])
            for i in range(NT):
                ps = ps_s_pool.tile([P, P], F16, tag="ps_s")
                nc.tensor.transpose(ps[:], q_r[:, i, :], ident_bf[:])
                nc.scalar.copy(qt_t[i][:], ps[:])

            v_f32 = big_pool.tile([P, N_CH, D], F32)

            def load_kt_chunk(mc):
                k_nat = stage_pool.tile([P, 4, P], F32, tag="nat")
                nc.sync.dma_start(
                    k_nat[:],
                    k[ds(mc * MC, MC), :].rearrange("(c p) d -> p c d", p=P),
                )
                k_r = stage_pool.tile([P, 4, P], F16, tag="natr")
                nc.vector.tensor_copy(k_r[:], k_nat[:])
                for b in range(4):
                    ps = ps_s_pool.tile([P, P], F16, tag="ps_s")
                    nc.tensor.transpose(ps[:], k_r[:, b, :], ident_bf[:])
                    nc.scalar.copy(kt_t[mc][:, ts(b, P)], ps[:])

            for mc0 in range(TGROUP):
                load_kt_chunk(mc0)

            # -- main loop --
            for nt in range(NT):
                ps_o = ps_o_pool.tile([P, D + 1], F32, tag="ps_o")
                for g in range(N_G):
                    if nt == 0:
                        # stage this group's V chunks: [V | ones] fp16
                        cs = slice(g * (GW // P), (g + 1) * (GW // P))
                        nc.sync.dma_start(
                            v_f32[:, cs, :],
                            v[ds(g * GW, GW), :].rearrange(
                                "(c p) d -> p c d", p=P
                            ),
                        )
                        nc.vector.tensor_copy(vaug[:, cs, 0:D], v_f32[:, cs, :])
                        nc.vector.memset(vaug[:, cs, D : D + 1], 1.0)
                    ps_t = ps_t_pool.tile([P, GW], F16)
                    for j in range(TGROUP):
                        mc = g * TGROUP + j
                        ps_s = ps_s_pool.tile([P, MC], F32, tag="ps_s")
                        nc.tensor.matmul(
                            ps_s[:],
                            qt_t[nt][:],
                            kt_t[mc][:],
                            start=True,
                            stop=True,
                        )
                        if j == 0:
                            m_tg = mask_pool.tile([P, GW], BF16, tag="m_tg")
                            nc.sync.dma_start(
                                m_tg[:], mask[ts(nt, P), ts(g, GW)]
                            )
                        m_t = m_tg[:, ds(j * MC, MC)]
                        sm = sm_pool.tile([P, MC], F16)
                        nc.vector.scalar_tensor_tensor(
                            sm[:], ps_s[:], SCALE, m_t, op0=MULT, op1=ADD
                        )
                        for b in range(MC // P):
                            nc.tensor.transpose(
                                ps_t[:, ds(j * MC + b * P, P)],
                                sm[:, ts(b, P)],
                                ident_bf[:],
                            )
                    p_t = pt_pool.tile([P, GW], F16)
                    nc.scalar.activation(
                        p_t[:], ps_t[:], mybir.ActivationFunctionType.Exp
                    )
                    for bb in range(GW // P):
                        cglob = g * (GW // P) + bb
                        nc.tensor.matmul(
                            ps_o[:],
                            p_t[:, ts(bb, P)],
                            vaug[:, cglob, :],
                            start=(cglob == 0),
                            stop=(cglob == N_CH - 1),
                        )
                    if nt == 0 and g + 1 < N_G:
                        # prefetch next group's K^T chunks; emitted after this
                        # group's compute so exp/stt never queue behind them
                        for j2 in range(TGROUP):
                            load_kt_chunk((g + 1) * TGROUP + j2)
                l_r = o_pool.tile([P, 1], F32, tag="lr")
                nc.vector.reciprocal(l_r[:], ps_o[:, D : D + 1])
                o_sb = o_pool.tile([P, D], F32, tag="osb")
                nc.vector.tensor_scalar(
                    o_sb[:], ps_o[:, 0:D], l_r[:], None, op0=MULT
                )
                nc.sync.dma_start(out[ts(nt, P), :], o_sb[:])

    nc.compile()
    return nc


_CACHE = {}


def _get_nc():
    if "nc" not in _CACHE:
        _CACHE["nc"] = build_nc()
    return _CACHE["nc"]


def _make_in_maps(q, k, v, mask):
    import ml_dtypes

    q = np.ascontiguousarray(np.asarray(q), dtype=np.float32)
    k = np.ascontiguousarray(np.asarray(k), dtype=np.float32)
    v = np.ascontiguousarray(np.asarray(v), dtype=np.float32)
    mask = np.asarray(mask)
    if mask.dtype != ml_dtypes.bfloat16:
        mask = mask.astype(ml_dtypes.bfloat16)
    in_maps = []
    for c in range(NCORES):
        sl = slice(c * N_SH, (c + 1) * N_SH)
        in_maps.append(
            {
                "q": q[sl],
                "k": k,
                "v": v,
                "mask": np.ascontiguousarray(mask[sl]),
            }
        )
    return in_maps


def _run(q, k, v, mask, **spmd_kwargs):
    nc = _get_nc()
    res = run_bass_kernel_spmd(
        nc, _make_in_maps(q, k, v, mask), core_ids=list(range(NCORES)), **spmd_kwargs
    )
    full = np.concatenate(
        [res.results[c]["out"] for c in range(NCORES)], axis=0
    ).astype(np.float32)
    return full, res


def kernel(q, k, v, mask):
    full, _ = _run(q, k, v, mask)
    return full
